# revision 13
# baseline (speedup 1.0000x reference)
"""Trainium2 Bass kernel for nn_CrossAttention (B=2, Lq=Lkv=2048, D=1024, H=16).

Sharding: 8 cores = 2 batches x 4 head-groups (4 heads / 256 dims each).
Per core (batch b, head group g):
  qhT[256,2048]  = (Wq_g * scale)^T-proj of q[b]      (head dims on partitions)
  kT  [256,2048] = Wk_g-proj of kv[b]
  v   [2048,256] = Wv_g-proj of kv[b], layernormed over each head's 64 dims
  per head h: attnT[kv,q] = sigmoid(kT_h^T-contract qhT_h + bias)  (transposed out)
              avT[64,q]  += v_h^T-contract attnT                   (psum accum over kv)
  proj[2048,1024] = avT^T @ WprojT_g (partial sum over this core's 256 dims)
Host: pre-transposes/slices inputs, folds hd^-0.5 * exp(attn_temp) into Wq,
      transposes attnT/avT back, concats heads, sums the 4 Wproj partials.

All matmuls in float32r (full PE rate, ~tf32 ingest rounding); vector math fp32.
"""
import sys

import numpy as np

sys.path.insert(0, "/opt/trn_rl_repo")

import concourse.mybir as mybir  # noqa: E402
import concourse.tile as tile  # noqa: E402
from concourse import bacc  # noqa: E402
from concourse.bass_utils import run_bass_kernel_spmd  # noqa: E402

F32 = mybir.dt.float32
F32R = mybir.dt.float32r
F16 = mybir.dt.float16
AF = mybir.ActivationFunctionType
ALU = mybir.AluOpType

B = 2
LQ = 2048
LKV = 2048
D = 1024
H = 16
HD = 64
NCORES = 8
HL = H // 4          # 4 local heads per core
DL = HL * HD         # 256 local head dims
LN_EPS = 1e-5

ATTN_F16 = False  # fp16 attn_matrix storage + fp16 attn@v operands (saves ~33MB/core DMA)

_cached = {}


def build_program(lq=LQ, lkv=LKV, attn_f16=None):
    if attn_f16 is None:
        attn_f16 = ATTN_F16
    sig_dt = F16 if attn_f16 else F32R
    out_dt = F16 if attn_f16 else F32
    nq = lq // 512       # q chunks of 512
    nqt = lq // 128      # q tiles of 128
    nkt = lkv // 128     # kv tiles of 128
    nds = D // 128       # contraction slabs over D
    HKV = lkv // 2       # input half width
    HQ = lq // 2

    nc = bacc.Bacc("TRN2", target_bir_lowering=False, debug=False)

    qT_d = nc.dram_tensor("qT", [D, lq], F32R, kind="ExternalInput")
    kvT_d = nc.dram_tensor("kvT", [D, lkv], F32R, kind="ExternalInput")
    wq_d = nc.dram_tensor("wqT", [D, DL], F32R, kind="ExternalInput")
    wk_d = nc.dram_tensor("wkT", [D, DL], F32R, kind="ExternalInput")
    wv_d = nc.dram_tensor("wvT", [D, DL], F32R, kind="ExternalInput")
    wp_d = nc.dram_tensor("wpT", [HL, HD, D], F32R, kind="ExternalInput")
    vnw_d = nc.dram_tensor("vnw", [128, DL], F32, kind="ExternalInput")
    vnb_d = nc.dram_tensor("vnb", [128, DL], F32, kind="ExternalInput")
    bias_d = nc.dram_tensor("bias", [128, 1], F32, kind="ExternalInput")

    attnT_d = nc.dram_tensor("attnT", [HL, lkv, lq], out_dt, kind="ExternalOutput")
    avT_d = nc.dram_tensor("avT", [DL, lq], F32, kind="ExternalOutput")
    proj_d = nc.dram_tensor("proj", [lq, D], F32, kind="ExternalOutput")

    with tile.TileContext(nc) as tc:
        with (
            tc.tile_pool(name="slabs", bufs=16) as slabs,
            tc.tile_pool(name="wts", bufs=1) as wts,
            tc.tile_pool(name="per", bufs=1) as per,
            tc.tile_pool(name="scr", bufs=2) as scr,
            tc.tile_pool(name="sm", bufs=4) as sm,
            tc.tile_pool(name="stg", bufs=2) as stg,
            tc.tile_pool(name="psB", bufs=2, space="PSUM") as psB,
        ):
            # ---- weights / constants ----
            wq = [wts.tile([128, DL], F32R, tag=f"wq{s}", name=f"wq{s}") for s in range(nds)]
            wk = [wts.tile([128, DL], F32R, tag=f"wk{s}", name=f"wk{s}") for s in range(nds)]
            wv = [wts.tile([128, DL], F32R, tag=f"wv{s}", name=f"wv{s}") for s in range(nds)]
            wp = [wts.tile([HD, D], F32R, tag=f"wp{h}", name=f"wp{h}") for h in range(HL)]
            vnw = wts.tile([128, DL], F32, tag="vnw")
            vnb = wts.tile([128, DL], F32, tag="vnb")
            bias = wts.tile([128, 1], F32, tag="bias")
            for s in range(nds):
                nc.sync.dma_start(wq[s][:], wq_d[s * 128:(s + 1) * 128, :])
            nc.sync.dma_start(bias[:], bias_d[:])

            def load_rest_weights():
                for s in range(nds):
                    nc.sync.dma_start(wk[s][:], wk_d[s * 128:(s + 1) * 128, :])
                    nc.sync.dma_start(wv[s][:], wv_d[s * 128:(s + 1) * 128, :])
                for h in range(HL):
                    nc.sync.dma_start(wp[h][:], wp_d[h])
                nc.sync.dma_start(vnw[:], vnw_d[:])
                nc.sync.dma_start(vnb[:], vnb_d[:])

            # ---- persistent activations ----
            qhT = [per.tile([128, lq], F32R, tag=f"qhT{m}", name=f"qhT{m}") for m in range(2)]
            kT = [per.tile([128, lkv], F32R, tag=f"kT{m}", name=f"kT{m}") for m in range(2)]
            vln = [per.tile([128, DL], sig_dt, tag=f"vln{t}", name=f"vln{t}") for t in range(nkt)]
            avT = [per.tile([HD, lq], F32R, tag=f"avT{h}", name=f"avT{h}") for h in range(HL)]

            # ---- phase 1a: qhT (q input streamed as [128,1024] halves) ----
            qts = [
                [slabs.tile([128, HQ], F32R, tag="slab", name=f"qts{half}_{s}")
                 for s in range(nds)]
                for half in range(2)
            ]
            kvts = [
                [slabs.tile([128, HKV], F32R, tag="slab", name=f"kvts{half}_{s}")
                 for s in range(nds)]
                for half in range(2)
            ]
            def load_q_half(half):
                for s in range(nds):
                    nc.sync.dma_start(
                        qts[half][s][:],
                        qT_d[s * 128:(s + 1) * 128, half * HQ:(half + 1) * HQ],
                    )

            def do_q_half(half):
                for m in range(2):
                    ps = psB.tile([128, 1024], F32, tag="mm", name=f"psq{half}_{m}")
                    for ci in range(2):
                        for s in range(nds):
                            nc.tensor.matmul(
                                ps[:, ci * 512:(ci + 1) * 512],
                                wq[s][:, m * 128:(m + 1) * 128],
                                qts[half][s][:, ci * 512:(ci + 1) * 512],
                                start=(s == 0), stop=(s == nds - 1),
                            )
                    nc.vector.tensor_copy(
                        qhT[m][:, half * HQ:(half + 1) * HQ], ps[:]
                    )

            # ---- phase 1b: kT and v (+layernorm) ----
            def load_kv_half(half):
                for s in range(nds):
                    nc.sync.dma_start(
                        kvts[half][s][:],
                        kvT_d[s * 128:(s + 1) * 128, half * HKV:(half + 1) * HKV],
                    )

            def do_k_half(half):
                for m in range(2):
                    ps = psB.tile([128, 1024], F32, tag="mm", name=f"psk{half}_{m}")
                    for ci in range(2):
                        for s in range(nds):
                            nc.tensor.matmul(
                                ps[:, ci * 512:(ci + 1) * 512],
                                wk[s][:, m * 128:(m + 1) * 128],
                                kvts[half][s][:, ci * 512:(ci + 1) * 512],
                                start=(s == 0), stop=(s == nds - 1),
                            )
                    nc.vector.tensor_copy(
                        kT[m][:, half * HKV:(half + 1) * HKV], ps[:]
                    )

            def do_v_tile(t):
                half, ti = t // (nkt // 2), t % (nkt // 2)
                pv = psB.tile([128, DL], F32, tag="av", bufs=4, name=f"pv{t}")
                for s in range(nds):
                    nc.tensor.matmul(
                        pv[:],
                        kvts[half][s][:, ti * 128:(ti + 1) * 128],
                        wv[s][:],
                        start=(s == 0), stop=(s == nds - 1),
                    )
                # layernorm over each head's 64 dims; sums on DVE
                sums = sm.tile([128, HL], F32, tag="sums", name=f"sums{t}")
                nc.vector.reduce_sum(
                    sums[:], pv[:].rearrange("p (h d) -> p h d", d=HD),
                    axis=mybir.AxisListType.X,
                )
                sq = scr.tile([128, DL], F32, tag="sq", name=f"sq{t}")
                nc.scalar.activation(sq[:], pv[:], AF.Square)
                ssq = sm.tile([128, HL], F32, tag="ssq", name=f"ssq{t}")
                nc.vector.reduce_sum(
                    ssq[:], sq[:].rearrange("p (h d) -> p h d", d=HD),
                    axis=mybir.AxisListType.X,
                )
                mu = sm.tile([128, HL], F32, tag="mu", name=f"mu{t}")
                nc.vector.tensor_scalar_mul(mu[:], sums[:], 1.0 / HD)
                mu2 = sm.tile([128, HL], F32, tag="mu2", name=f"mu2{t}")
                nc.vector.tensor_mul(mu2[:], mu[:], mu[:])
                vv = sm.tile([128, HL], F32, tag="vv", name=f"vv{t}")
                # vv = ssq/HD + eps - mu^2
                nc.vector.tensor_scalar(
                    vv[:], ssq[:], 1.0 / HD, LN_EPS, op0=ALU.mult, op1=ALU.add,
                )
                nc.vector.tensor_sub(vv[:], vv[:], mu2[:])
                std = sm.tile([128, HL], F32, tag="std", name=f"std{t}")
                nc.scalar.activation(std[:], vv[:], AF.Sqrt)
                rstd = sm.tile([128, HL], F32, tag="rstd", name=f"rstd{t}")
                nc.vector.reciprocal(rstd[:], std[:])
                xn = scr.tile([128, DL], F32, tag="xn", name=f"xn{t}")
                for h in range(HL):
                    nc.vector.tensor_scalar(
                        xn[:, h * HD:(h + 1) * HD], pv[:, h * HD:(h + 1) * HD],
                        mu[:, h:h + 1], rstd[:, h:h + 1],
                        op0=ALU.subtract, op1=ALU.mult,
                    )
                t2 = scr.tile([128, DL], F32, tag="t2", name=f"t2{t}")
                nc.vector.tensor_mul(t2[:], xn[:], vnw[:])
                nc.vector.tensor_add(vln[t][:], t2[:], vnb[:])

            # A halves first; B loads overlap A compute as slots free
            load_q_half(0)
            load_kv_half(0)
            load_rest_weights()
            do_q_half(0)
            load_q_half(1)
            do_k_half(0)
            load_kv_half(1)
            do_q_half(1)
            for t in range(nkt // 2):
                do_v_tile(t)

            # ---- phase 2: per head: logits -> sigmoid -> attnT + avT accum ----
            avps_all = {}

            def do_head(h, t0, t1):
                pair, pofs = h // 2, 64 * (h % 2)
                if h not in avps_all:
                    avps_all[h] = [
                        psB.tile([HD, 512], F32, tag="av", bufs=4, name=f"avps{h}_{c}")
                        for c in range(nq)
                    ]
                avps = avps_all[h]
                for t in range(t0, t1):
                    for half in range(2):
                        sig = slabs.tile(
                            [128, HQ], sig_dt, tag="slab", name=f"sig{h}_{t}_{half}"
                        )
                        pl = psB.tile(
                            [128, 1024], F32, tag="mm", name=f"pl{h}_{t}_{half}"
                        )
                        for ci in range(2):
                            c = half * 2 + ci
                            nc.tensor.matmul(
                                pl[:, ci * 512:(ci + 1) * 512],
                                kT[pair][pofs:pofs + 64, t * 128:(t + 1) * 128],
                                qhT[pair][pofs:pofs + 64, c * 512:(c + 1) * 512],
                                start=True, stop=True,
                            )
                        nc.scalar.activation(
                            sig[:], pl[:], AF.Sigmoid, bias=bias[:], scale=1.0,
                        )
                        for ci in range(2):
                            c = half * 2 + ci
                            nc.tensor.matmul(
                                avps[c][:],
                                vln[t][:, h * HD:(h + 1) * HD],
                                sig[:, ci * 512:(ci + 1) * 512],
                                start=(t == 0), stop=(t == nkt - 1),
                            )
                        nc.sync.dma_start(
                            attnT_d[h, t * 128:(t + 1) * 128, half * HQ:(half + 1) * HQ],
                            sig[:] if attn_f16 else sig[:].bitcast(F32),
                        )
                if t1 == nkt:
                    for c in range(nq):
                        nc.vector.tensor_copy(
                            avT[h][:, c * 512:(c + 1) * 512], avps[c][:]
                        )
                    nc.sync.dma_start(
                        avT_d[h * HD:(h + 1) * HD, :], avT[h][:].bitcast(F32)
                    )

            do_k_half(1)
            for t in range(nkt // 2, nkt):
                do_v_tile(t)
            for h in range(HL):
                do_head(h, 0, nkt)

            # ---- phase 3: proj partial = avT^T @ wpT ----
            for t in range(nqt):
                pp = psB.tile([128, 1024], F32, tag="mm", name=f"pp{t}")
                for oc in range(2):
                    for h in range(HL):
                        nc.tensor.matmul(
                            pp[:, oc * 512:(oc + 1) * 512],
                            avT[h][:, t * 128:(t + 1) * 128],
                            wp[h][:, oc * 512:(oc + 1) * 512],
                            start=(h == 0), stop=(h == HL - 1),
                        )
                st = stg.tile([128, 1024], F32, tag="st", name=f"st{t}")
                nc.vector.tensor_copy(st[:], pp[:])
                nc.sync.dma_start(proj_d[t * 128:(t + 1) * 128, :], st[:])

    nc.finalize()
    return nc


def _prep_inputs(q, kv, Wq, Wkv, Wproj, vn_w, vn_b, attn_temp, attn_bias):
    q = np.asarray(q, np.float32)
    kv = np.asarray(kv, np.float32)
    Wq = np.asarray(Wq, np.float32)
    Wkv = np.asarray(Wkv, np.float32)
    Wproj = np.asarray(Wproj, np.float32)
    vn_w = np.asarray(vn_w, np.float32)
    vn_b = np.asarray(vn_b, np.float32)
    scale = np.float64(HD) ** -0.5 * np.exp(np.float64(np.asarray(attn_temp)[0]))
    bias = np.float32(np.asarray(attn_bias)[0])

    vnw_rep = np.ascontiguousarray(np.broadcast_to(np.tile(vn_w, HL)[None, :], (128, DL)))
    vnb_rep = np.ascontiguousarray(np.broadcast_to(np.tile(vn_b, HL)[None, :], (128, DL)))
    bias_col = np.full((128, 1), bias, np.float32)

    qT = [np.ascontiguousarray(q[b].T) for b in range(B)]
    kvT = [np.ascontiguousarray(kv[b].T) for b in range(B)]

    in_maps = []
    for c in range(NCORES):
        b, g = c // 4, c % 4
        sl = slice(g * DL, (g + 1) * DL)
        wqT = np.ascontiguousarray((Wq[sl, :] * np.float32(scale)).T)
        wkT = np.ascontiguousarray(Wkv[sl, :].T)
        wvT = np.ascontiguousarray(Wkv[D + g * DL:D + (g + 1) * DL, :].T)
        wpT = np.ascontiguousarray(Wproj[:, sl].T).reshape(HL, HD, D)
        in_maps.append({
            "qT": qT[b], "kvT": kvT[b],
            "wqT": wqT, "wkT": wkT, "wvT": wvT, "wpT": wpT,
            "vnw": vnw_rep, "vnb": vnb_rep, "bias": bias_col,
        })
    return in_maps


def run_sharded(in_maps, **kwargs):
    if "nc" not in _cached:
        _cached["nc"] = build_program()
    return run_bass_kernel_spmd(
        _cached["nc"], in_maps, core_ids=list(range(NCORES)), **kwargs
    )


def _unshard(results):
    attn_matrix = np.empty((B, H, LQ, LKV), np.float32)
    attn_times_v = np.empty((B, LQ, D), np.float32)
    attn_proj = np.zeros((B, LQ, D), np.float32)
    for c in range(NCORES):
        b, g = c // 4, c % 4
        r = results[c]
        at = r["attnT"]  # [HL, lkv, lq]
        for h in range(HL):
            attn_matrix[b, g * HL + h] = at[h].T.astype(np.float32)
        attn_times_v[b, :, g * DL:(g + 1) * DL] = r["avT"].T  # [DL, lq] -> [lq, DL]
        attn_proj[b] += r["proj"]
    return attn_matrix, attn_times_v, attn_proj


def kernel(q, kv, Wq, Wkv, Wproj, vn_w, vn_b, attn_temp, attn_bias):
    in_maps = _prep_inputs(q, kv, Wq, Wkv, Wproj, vn_w, vn_b, attn_temp, attn_bias)
    res = run_sharded(in_maps)
    return _unshard(res.results)


# revision 15
# speedup vs baseline: 1.0006x; 1.0006x over previous
"""Trainium2 Bass kernel for nn_CrossAttention (B=2, Lq=Lkv=2048, D=1024, H=16).

Sharding: 8 cores = 2 batches x 4 head-groups (4 heads / 256 dims each).
Per core (batch b, head group g):
  qhT[256,2048]  = (Wq_g * scale)^T-proj of q[b]      (head dims on partitions)
  kT  [256,2048] = Wk_g-proj of kv[b]
  v   [2048,256] = Wv_g-proj of kv[b], layernormed over each head's 64 dims
  per head h: attnT[kv,q] = sigmoid(kT_h^T-contract qhT_h + bias)  (transposed out)
              avT[64,q]  += v_h^T-contract attnT                   (psum accum over kv)
  proj[2048,1024] = avT^T @ WprojT_g (partial sum over this core's 256 dims)
Host: pre-transposes/slices inputs, folds hd^-0.5 * exp(attn_temp) into Wq,
      transposes attnT/avT back, concats heads, sums the 4 Wproj partials.

All matmuls in float32r (full PE rate, ~tf32 ingest rounding); vector math fp32.
"""
import sys

import numpy as np

sys.path.insert(0, "/opt/trn_rl_repo")

import concourse.mybir as mybir  # noqa: E402
import concourse.tile as tile  # noqa: E402
from concourse import bacc  # noqa: E402
from concourse.bass_utils import run_bass_kernel_spmd  # noqa: E402

F32 = mybir.dt.float32
F32R = mybir.dt.float32r
F16 = mybir.dt.float16
AF = mybir.ActivationFunctionType
ALU = mybir.AluOpType

B = 2
LQ = 2048
LKV = 2048
D = 1024
H = 16
HD = 64
NCORES = 8
HL = H // 4          # 4 local heads per core
DL = HL * HD         # 256 local head dims
LN_EPS = 1e-5

ATTN_F16 = False  # fp16 attn_matrix storage + fp16 attn@v operands (saves ~33MB/core DMA)

_cached = {}


def build_program(lq=LQ, lkv=LKV, attn_f16=None):
    if attn_f16 is None:
        attn_f16 = ATTN_F16
    sig_dt = F16 if attn_f16 else F32R
    out_dt = F16 if attn_f16 else F32
    nq = lq // 512       # q chunks of 512
    nqt = lq // 128      # q tiles of 128
    nkt = lkv // 128     # kv tiles of 128
    nds = D // 128       # contraction slabs over D
    HKV = lkv // 2       # input half width
    HQ = lq // 2

    nc = bacc.Bacc("TRN2", target_bir_lowering=False, debug=False)

    qT_d = nc.dram_tensor("qT", [D, lq], F32R, kind="ExternalInput")
    kvT_d = nc.dram_tensor("kvT", [D, lkv], F32R, kind="ExternalInput")
    wq_d = nc.dram_tensor("wqT", [D, DL], F32R, kind="ExternalInput")
    wk_d = nc.dram_tensor("wkT", [D, DL], F32R, kind="ExternalInput")
    wv_d = nc.dram_tensor("wvT", [D, DL], F32R, kind="ExternalInput")
    wp_d = nc.dram_tensor("wpT", [HL, HD, D], F32R, kind="ExternalInput")
    vnw_d = nc.dram_tensor("vnw", [128, DL], F32, kind="ExternalInput")
    vnb_d = nc.dram_tensor("vnb", [128, DL], F32, kind="ExternalInput")
    bias_d = nc.dram_tensor("bias", [128, 1], F32, kind="ExternalInput")

    attnT_d = nc.dram_tensor("attnT", [HL, lkv, lq], out_dt, kind="ExternalOutput")
    avT_d = nc.dram_tensor("avT", [DL, lq], F32, kind="ExternalOutput")
    proj_d = nc.dram_tensor("proj", [lq, D], F32, kind="ExternalOutput")

    with tile.TileContext(nc) as tc:
        with (
            tc.tile_pool(name="slabs", bufs=16) as slabs,
            tc.tile_pool(name="wts", bufs=1) as wts,
            tc.tile_pool(name="per", bufs=1) as per,
            tc.tile_pool(name="scr", bufs=2) as scr,
            tc.tile_pool(name="sm", bufs=4) as sm,
            tc.tile_pool(name="stg", bufs=2) as stg,
            tc.tile_pool(name="psB", bufs=2, space="PSUM") as psB,
        ):
            # ---- weights / constants ----
            wq = [wts.tile([128, DL], F32R, tag=f"wq{s}", name=f"wq{s}") for s in range(nds)]
            wk = [wts.tile([128, DL], F32R, tag=f"wk{s}", name=f"wk{s}") for s in range(nds)]
            wv = [wts.tile([128, DL], F32R, tag=f"wv{s}", name=f"wv{s}") for s in range(nds)]
            wp = [wts.tile([HD, D], F32R, tag=f"wp{h}", name=f"wp{h}") for h in range(HL)]
            vnw = wts.tile([128, DL], F32, tag="vnw")
            vnb = wts.tile([128, DL], F32, tag="vnb")
            bias = wts.tile([128, 1], F32, tag="bias")
            for s in range(nds):
                nc.sync.dma_start(wq[s][:], wq_d[s * 128:(s + 1) * 128, :])
            nc.sync.dma_start(bias[:], bias_d[:])

            def load_rest_weights():
                for s in range(nds):
                    nc.sync.dma_start(wk[s][:], wk_d[s * 128:(s + 1) * 128, :])
                    nc.sync.dma_start(wv[s][:], wv_d[s * 128:(s + 1) * 128, :])
                for h in range(HL):
                    nc.sync.dma_start(wp[h][:], wp_d[h])
                nc.sync.dma_start(vnw[:], vnw_d[:])
                nc.sync.dma_start(vnb[:], vnb_d[:])

            # ---- persistent activations ----
            qhT = [per.tile([128, lq], F32R, tag=f"qhT{m}", name=f"qhT{m}") for m in range(2)]
            kT = [per.tile([128, lkv], F32R, tag=f"kT{m}", name=f"kT{m}") for m in range(2)]
            vln = [per.tile([128, DL], sig_dt, tag=f"vln{t}", name=f"vln{t}") for t in range(nkt)]
            avT = [per.tile([HD, lq], F32R, tag=f"avT{h}", name=f"avT{h}") for h in range(HL)]

            # ---- phase 1a: qhT (q input streamed as [128,1024] halves) ----
            qts = [
                [slabs.tile([128, HQ], F32R, tag="slab", name=f"qts{half}_{s}")
                 for s in range(nds)]
                for half in range(2)
            ]
            kvts = [
                [slabs.tile([128, HKV], F32R, tag="slab", name=f"kvts{half}_{s}")
                 for s in range(nds)]
                for half in range(2)
            ]
            def load_q_half(half):
                for s in range(nds):
                    nc.sync.dma_start(
                        qts[half][s][:],
                        qT_d[s * 128:(s + 1) * 128, half * HQ:(half + 1) * HQ],
                    )

            def do_q_half(half):
                for m in range(2):
                    ps = psB.tile([128, 1024], F32, tag="mm", name=f"psq{half}_{m}")
                    for s in range(nds):
                        for ci in range(2):
                            nc.tensor.matmul(
                                ps[:, ci * 512:(ci + 1) * 512],
                                wq[s][:, m * 128:(m + 1) * 128],
                                qts[half][s][:, ci * 512:(ci + 1) * 512],
                                start=(s == 0), stop=(s == nds - 1),
                            )
                    nc.vector.tensor_copy(
                        qhT[m][:, half * HQ:(half + 1) * HQ], ps[:]
                    )

            # ---- phase 1b: kT and v (+layernorm) ----
            def load_kv_half(half):
                for s in range(nds):
                    nc.sync.dma_start(
                        kvts[half][s][:],
                        kvT_d[s * 128:(s + 1) * 128, half * HKV:(half + 1) * HKV],
                    )

            def do_k_half(half):
                for m in range(2):
                    ps = psB.tile([128, 1024], F32, tag="mm", name=f"psk{half}_{m}")
                    for s in range(nds):
                        for ci in range(2):
                            nc.tensor.matmul(
                                ps[:, ci * 512:(ci + 1) * 512],
                                wk[s][:, m * 128:(m + 1) * 128],
                                kvts[half][s][:, ci * 512:(ci + 1) * 512],
                                start=(s == 0), stop=(s == nds - 1),
                            )
                    nc.vector.tensor_copy(
                        kT[m][:, half * HKV:(half + 1) * HKV], ps[:]
                    )

            def do_v_tile(t):
                half, ti = t // (nkt // 2), t % (nkt // 2)
                pv = psB.tile([128, DL], F32, tag="av", bufs=4, name=f"pv{t}")
                for s in range(nds):
                    nc.tensor.matmul(
                        pv[:],
                        kvts[half][s][:, ti * 128:(ti + 1) * 128],
                        wv[s][:],
                        start=(s == 0), stop=(s == nds - 1),
                    )
                # layernorm over each head's 64 dims; sums on DVE
                sums = sm.tile([128, HL], F32, tag="sums", name=f"sums{t}")
                nc.vector.reduce_sum(
                    sums[:], pv[:].rearrange("p (h d) -> p h d", d=HD),
                    axis=mybir.AxisListType.X,
                )
                sq = scr.tile([128, DL], F32, tag="sq", name=f"sq{t}")
                nc.scalar.activation(sq[:], pv[:], AF.Square)
                ssq = sm.tile([128, HL], F32, tag="ssq", name=f"ssq{t}")
                nc.vector.reduce_sum(
                    ssq[:], sq[:].rearrange("p (h d) -> p h d", d=HD),
                    axis=mybir.AxisListType.X,
                )
                mu = sm.tile([128, HL], F32, tag="mu", name=f"mu{t}")
                nc.vector.tensor_scalar_mul(mu[:], sums[:], 1.0 / HD)
                mu2 = sm.tile([128, HL], F32, tag="mu2", name=f"mu2{t}")
                nc.vector.tensor_mul(mu2[:], mu[:], mu[:])
                vv = sm.tile([128, HL], F32, tag="vv", name=f"vv{t}")
                # vv = ssq/HD + eps - mu^2
                nc.vector.tensor_scalar(
                    vv[:], ssq[:], 1.0 / HD, LN_EPS, op0=ALU.mult, op1=ALU.add,
                )
                nc.vector.tensor_sub(vv[:], vv[:], mu2[:])
                std = sm.tile([128, HL], F32, tag="std", name=f"std{t}")
                nc.scalar.activation(std[:], vv[:], AF.Sqrt)
                rstd = sm.tile([128, HL], F32, tag="rstd", name=f"rstd{t}")
                nc.vector.reciprocal(rstd[:], std[:])
                xn = scr.tile([128, DL], F32, tag="xn", name=f"xn{t}")
                for h in range(HL):
                    nc.vector.tensor_scalar(
                        xn[:, h * HD:(h + 1) * HD], pv[:, h * HD:(h + 1) * HD],
                        mu[:, h:h + 1], rstd[:, h:h + 1],
                        op0=ALU.subtract, op1=ALU.mult,
                    )
                t2 = scr.tile([128, DL], F32, tag="t2", name=f"t2{t}")
                nc.vector.tensor_mul(t2[:], xn[:], vnw[:])
                nc.vector.tensor_add(vln[t][:], t2[:], vnb[:])

            # A halves first; B loads overlap A compute as slots free
            load_q_half(0)
            load_kv_half(0)
            load_rest_weights()
            do_q_half(0)
            load_q_half(1)
            do_k_half(0)
            load_kv_half(1)
            do_q_half(1)
            for t in range(nkt // 2):
                do_v_tile(t)

            # ---- phase 2: per head: logits -> sigmoid -> attnT + avT accum ----
            avps_all = {}

            def do_head(h, t0, t1):
                pair, pofs = h // 2, 64 * (h % 2)
                if h not in avps_all:
                    avps_all[h] = [
                        psB.tile([HD, 512], F32, tag="av", bufs=4, name=f"avps{h}_{c}")
                        for c in range(nq)
                    ]
                avps = avps_all[h]
                for t in range(t0, t1):
                    for half in range(2):
                        sig = slabs.tile(
                            [128, HQ], sig_dt, tag="slab", name=f"sig{h}_{t}_{half}"
                        )
                        pl = psB.tile(
                            [128, 1024], F32, tag="mm", name=f"pl{h}_{t}_{half}"
                        )
                        for ci in range(2):
                            c = half * 2 + ci
                            nc.tensor.matmul(
                                pl[:, ci * 512:(ci + 1) * 512],
                                kT[pair][pofs:pofs + 64, t * 128:(t + 1) * 128],
                                qhT[pair][pofs:pofs + 64, c * 512:(c + 1) * 512],
                                start=True, stop=True,
                            )
                        nc.scalar.activation(
                            sig[:], pl[:], AF.Sigmoid, bias=bias[:], scale=1.0,
                        )
                        for ci in range(2):
                            c = half * 2 + ci
                            nc.tensor.matmul(
                                avps[c][:],
                                vln[t][:, h * HD:(h + 1) * HD],
                                sig[:, ci * 512:(ci + 1) * 512],
                                start=(t == 0), stop=(t == nkt - 1),
                            )
                        nc.sync.dma_start(
                            attnT_d[h, t * 128:(t + 1) * 128, half * HQ:(half + 1) * HQ],
                            sig[:] if attn_f16 else sig[:].bitcast(F32),
                        )
                if t1 == nkt:
                    for c in range(nq):
                        nc.vector.tensor_copy(
                            avT[h][:, c * 512:(c + 1) * 512], avps[c][:]
                        )
                    nc.sync.dma_start(
                        avT_d[h * HD:(h + 1) * HD, :], avT[h][:].bitcast(F32)
                    )

            do_k_half(1)
            for t in range(nkt // 2, nkt):
                do_v_tile(t)
            for h in range(HL):
                do_head(h, 0, nkt)

            # ---- phase 3: proj partial = avT^T @ wpT ----
            for t in range(nqt):
                pp = psB.tile([128, 1024], F32, tag="mm", name=f"pp{t}")
                for oc in range(2):
                    for h in range(HL):
                        nc.tensor.matmul(
                            pp[:, oc * 512:(oc + 1) * 512],
                            avT[h][:, t * 128:(t + 1) * 128],
                            wp[h][:, oc * 512:(oc + 1) * 512],
                            start=(h == 0), stop=(h == HL - 1),
                        )
                st = stg.tile([128, 1024], F32, tag="st", name=f"st{t}")
                nc.vector.tensor_copy(st[:], pp[:])
                nc.sync.dma_start(proj_d[t * 128:(t + 1) * 128, :], st[:])

    nc.finalize()
    return nc


def _prep_inputs(q, kv, Wq, Wkv, Wproj, vn_w, vn_b, attn_temp, attn_bias):
    q = np.asarray(q, np.float32)
    kv = np.asarray(kv, np.float32)
    Wq = np.asarray(Wq, np.float32)
    Wkv = np.asarray(Wkv, np.float32)
    Wproj = np.asarray(Wproj, np.float32)
    vn_w = np.asarray(vn_w, np.float32)
    vn_b = np.asarray(vn_b, np.float32)
    scale = np.float64(HD) ** -0.5 * np.exp(np.float64(np.asarray(attn_temp)[0]))
    bias = np.float32(np.asarray(attn_bias)[0])

    vnw_rep = np.ascontiguousarray(np.broadcast_to(np.tile(vn_w, HL)[None, :], (128, DL)))
    vnb_rep = np.ascontiguousarray(np.broadcast_to(np.tile(vn_b, HL)[None, :], (128, DL)))
    bias_col = np.full((128, 1), bias, np.float32)

    qT = [np.ascontiguousarray(q[b].T) for b in range(B)]
    kvT = [np.ascontiguousarray(kv[b].T) for b in range(B)]

    in_maps = []
    for c in range(NCORES):
        b, g = c // 4, c % 4
        sl = slice(g * DL, (g + 1) * DL)
        wqT = np.ascontiguousarray((Wq[sl, :] * np.float32(scale)).T)
        wkT = np.ascontiguousarray(Wkv[sl, :].T)
        wvT = np.ascontiguousarray(Wkv[D + g * DL:D + (g + 1) * DL, :].T)
        wpT = np.ascontiguousarray(Wproj[:, sl].T).reshape(HL, HD, D)
        in_maps.append({
            "qT": qT[b], "kvT": kvT[b],
            "wqT": wqT, "wkT": wkT, "wvT": wvT, "wpT": wpT,
            "vnw": vnw_rep, "vnb": vnb_rep, "bias": bias_col,
        })
    return in_maps


def run_sharded(in_maps, **kwargs):
    if "nc" not in _cached:
        _cached["nc"] = build_program()
    return run_bass_kernel_spmd(
        _cached["nc"], in_maps, core_ids=list(range(NCORES)), **kwargs
    )


def _unshard(results):
    attn_matrix = np.empty((B, H, LQ, LKV), np.float32)
    attn_times_v = np.empty((B, LQ, D), np.float32)
    attn_proj = np.zeros((B, LQ, D), np.float32)
    for c in range(NCORES):
        b, g = c // 4, c % 4
        r = results[c]
        at = r["attnT"]  # [HL, lkv, lq]
        for h in range(HL):
            attn_matrix[b, g * HL + h] = at[h].T.astype(np.float32)
        attn_times_v[b, :, g * DL:(g + 1) * DL] = r["avT"].T  # [DL, lq] -> [lq, DL]
        attn_proj[b] += r["proj"]
    return attn_matrix, attn_times_v, attn_proj


def kernel(q, kv, Wq, Wkv, Wproj, vn_w, vn_b, attn_temp, attn_bias):
    in_maps = _prep_inputs(q, kv, Wq, Wkv, Wproj, vn_w, vn_b, attn_temp, attn_bias)
    res = run_sharded(in_maps)
    return _unshard(res.results)
